# revision 17
# baseline (speedup 1.0000x reference)
"""Category-specific linear on 8 trn2 cores — hidden-dim sharding, resident W.

v3 (from v1 @488us, v2 @518us):
- bf16 output (halves out DMA + DVE bytes; total rel err ~3.7e-3 vs 2e-2).
- Out DMAs issued from the idle GpSimd queue -> their copy-done waits no
  longer block x/W prefetch issues on the Sync queue.
- Groups processed LARGEST category first: the first W tile covers the
  most compute, so the slow first ~25us of DMA (subsystem ramp) carries
  almost only x; small groups run late when the pipe is warm.
- All x/W DMAs on one Sync queue in exact need order; W tiles rotate
  through an 8-buffer pool (issues self-pace via buffer-reuse waits).
- First x pair + first W arrive as interleaved k-chunks so compute can
  start as soon as ~0.8MiB lands.
- Last sample: per-m-tile copies + DMAs to shrink the tail drain.
"""

import numpy as np
import ml_dtypes

B = 64
S = 512
DIN = 1024
DH = 4096
C = 16
NCORES = 8
NSH = DH // NCORES   # 512
P = 128
KO = DIN // P        # 8
MO = S // P          # 4

LAST_RESULTS = None


def _plan(cats):
    """Largest-first category order; proc_order[i] = original sample idx."""
    first = {}
    for j, c in enumerate(cats):
        first.setdefault(c, j)
    counts = {c: cats.count(c) for c in first}
    used = sorted(first, key=lambda c: (-counts[c], first[c]))
    proc_order = [j for c in used for j in range(len(cats)) if cats[j] == c]
    group_start = {}
    for i, j in enumerate(proc_order):
        group_start.setdefault(cats[j], i)
    return used, proc_order, group_start


def _build_program(cats):
    import concourse.bacc as bacc
    import concourse.mybir as mybir
    import concourse.tile as tile

    nc = bacc.Bacc("TRN2", target_bir_lowering=False)

    assert B % 4 == 0
    x_d = nc.dram_tensor("x", (B // 4, P, 4, KO, S), mybir.dt.bfloat16,
                         kind="ExternalInput")
    w_d = nc.dram_tensor("w", (C, P, KO, NSH), mybir.dt.bfloat16,
                         kind="ExternalInput")
    out_d = nc.dram_tensor("out", (B, P, MO, NSH), mybir.dt.bfloat16,
                           kind="ExternalOutput")

    used, proc_order, group_start = _plan(cats)
    n_groups = len(used)

    # k-chunking for the first x pair and group-0 W: compute starts as
    # soon as the first ~0.4MiB lands (DMA subsystem ramps slowly).
    CHUNKS = [(0, 1), (1, 2), (2, 4), (4, 6), (6, 8)]

    with tile.TileContext(nc) as tc:
        with (
            tc.tile_pool(name="static", bufs=1) as wp,
            tc.tile_pool(name="wgroup", bufs=6) as wg,
            tc.tile_pool(name="xpool", bufs=3) as xb,
            tc.tile_pool(name="opool", bufs=3) as ob,
            tc.tile_pool(name="psum", bufs=2, space="PSUM") as ps,
        ):
            w_tiles = {}

            def emit_w(g):
                c = used[g]
                t = wg.tile([P, KO, NSH], mybir.dt.bfloat16, tag="w")
                nc.sync.dma_start(t[:], w_d[c])
                w_tiles[c] = t

            # PE warmup: dummy matmuls with no DMA deps, run during the
            # ~10us DMA-subsystem init so the clock ramps before real data.
            warm_l = wp.tile([P, P], mybir.dt.bfloat16, tag="warm_l")
            warm_r = wp.tile([P, NSH], mybir.dt.bfloat16, tag="warm_r")
            nc.any.memzero(warm_l[:])
            nc.any.memzero(warm_r[:])
            warm_p = ps.tile([P, MO, NSH], mybir.dt.float32, tag="ps")
            for _ in range(5):
                nc.tensor.matmul(warm_p[:, 0, :], warm_l[:], warm_r[:],
                                 start=True, stop=True)

            # Head DMA sequence: interleaved k-chunks of (x samples 0-1,
            # W c0), in exact need order on one queue.
            c0 = used[0]
            x0c = []
            w0c = []
            for ci, (a, b) in enumerate(CHUNKS):
                tx = wp.tile([P, 2, b - a, S], mybir.dt.bfloat16,
                             tag=f"x0c{ci}")
                nc.sync.dma_start(tx[:], x_d[0, :, 0:2, a:b, :])
                x0c.append(tx)
                tw = wp.tile([P, b - a, NSH], mybir.dt.bfloat16,
                             tag=f"w0c{ci}")
                nc.sync.dma_start(tw[:], w_d[c0, :, a:b, :])
                w0c.append(tw)

            def chunk_of(k):
                for ci, (a, b) in enumerate(CHUNKS):
                    if a <= k < b:
                        return ci, k - a
                raise AssertionError

            # Samples 2-3 (second half of quad 0), then whole quads.
            x0b = wp.tile([P, 2, KO, S], mybir.dt.bfloat16, tag="x0b")
            nc.sync.dma_start(x0b[:], x_d[0, :, 2:4, :, :])

            xtiles = {}

            def emit_x(q):
                t = xb.tile([P, 4, KO, S], mybir.dt.bfloat16, tag="x")
                nc.sync.dma_start(t[:], x_d[q])
                xtiles[q] = t

            emit_x(1)

            # W group issue sample: 4 samples before first use.
            w_sched = {}
            for g in range(1, n_groups):
                w_sched.setdefault(max(0, group_start[used[g]] - 4), []).append(g)

            for i in range(B):
                j = proc_order[i]
                c = cats[j]
                for g in w_sched.get(i, ()):
                    emit_w(g)
                if i % 4 == 0:
                    q = i // 4 + 2
                    if q < B // 4:
                        emit_x(q)
                pt = ps.tile([P, MO, NSH], mybir.dt.float32, tag="ps")
                ot = ob.tile([P, MO, NSH], mybir.dt.bfloat16, tag="o")
                for m in range(MO):
                    for k in range(KO):
                        if i < 2:
                            ci, kk = chunk_of(k)
                            lhs = x0c[ci][:, i, kk, m * P:(m + 1) * P]
                        elif i < 4:
                            lhs = x0b[:, i - 2, k, m * P:(m + 1) * P]
                        else:
                            lhs = xtiles[i // 4][:, i % 4, k, m * P:(m + 1) * P]
                        if c == c0:
                            ci, kk = chunk_of(k)
                            rhs = w0c[ci][:, kk, :]
                        else:
                            rhs = w_tiles[c][:, k, :]
                        nc.tensor.matmul(
                            pt[:, m, :],
                            lhs,
                            rhs,
                            start=(k == 0),
                            stop=(k == KO - 1),
                        )
                if i == B - 1:
                    # Tail drain: independent half-tiles so the two copies
                    # run truly parallel (same-tile writes serialize), DMAs
                    # on Sync's wide queue.
                    ota = wp.tile([P, 2, NSH], mybir.dt.bfloat16, tag="ota")
                    otb = wp.tile([P, 2, NSH], mybir.dt.bfloat16, tag="otb")
                    nc.scalar.copy(ota[:], pt[:, 0:2, :])
                    nc.vector.tensor_copy(otb[:], pt[:, 2:4, :])
                    nc.sync.dma_start(out_d[i, :, 0:2, :], ota[:])
                    nc.sync.dma_start(out_d[i, :, 2:4, :], otb[:])
                else:
                    # Copies ride the otherwise-idle Scalar (ACT) engine.
                    nc.scalar.copy(ot[:], pt[:])
                    nc.gpsimd.dma_start(out_d[i], ot[:])

    nc.compile()
    return nc


def kernel(x, cat_ids, W, b):
    global LAST_RESULTS
    from concourse import bass_utils

    x = np.asarray(x, dtype=np.float32)
    cat_ids_np = np.asarray(cat_ids).astype(np.int64)
    W = np.asarray(W, dtype=np.float32)
    b = np.asarray(b, dtype=np.float32)
    cats = [int(c) for c in cat_ids_np]

    used, proc_order, _ = _plan(cats)

    # x: [B,S,DIN] -> [B,P(q),KO,S] bf16, in processed order, quadded.
    xp = np.ascontiguousarray(
        x.reshape(B, S, KO, P).transpose(0, 3, 2, 1)
    ).astype(ml_dtypes.bfloat16)[proc_order]
    xp = np.ascontiguousarray(
        xp.reshape(B // 4, 4, P, KO, S).transpose(0, 2, 1, 3, 4)
    )

    in_maps = []
    for core in range(NCORES):
        Wc = W[:, :, core * NSH:(core + 1) * NSH]
        Wp = np.ascontiguousarray(
            Wc.reshape(C, KO, P, NSH).transpose(0, 2, 1, 3)
        ).astype(ml_dtypes.bfloat16)
        in_maps.append({"x": xp, "w": Wp})

    nc = _build_program(cats)
    res = bass_utils.run_bass_kernel_spmd(
        nc, in_maps, core_ids=list(range(NCORES))
    )
    LAST_RESULTS = res

    inv = np.argsort(np.asarray(proc_order))
    out = np.empty((B, S, DH), dtype=np.float32)
    for core in range(NCORES):
        oc = np.asarray(res.results[core]["out"]).astype(np.float32)
        # out[proc_order[i], m*128+p, n] = oc[i, p, m, n]
        oc = oc.reshape(B, P, MO, NSH).transpose(0, 2, 1, 3).reshape(B, S, NSH)
        out[:, :, core * NSH:(core + 1) * NSH] = oc[inv]

    if b.any():
        out += b[cats][:, None, :]
    return out


# revision 20
# speedup vs baseline: 1.0107x; 1.0107x over previous
"""Category-specific linear on 8 trn2 cores — hidden-dim sharding, resident W.

v3 (from v1 @488us, v2 @518us):
- bf16 output (halves out DMA + DVE bytes; total rel err ~3.7e-3 vs 2e-2).
- Out DMAs issued from the idle GpSimd queue -> their copy-done waits no
  longer block x/W prefetch issues on the Sync queue.
- Groups processed LARGEST category first: the first W tile covers the
  most compute, so the slow first ~25us of DMA (subsystem ramp) carries
  almost only x; small groups run late when the pipe is warm.
- All x/W DMAs on one Sync queue in exact need order; W tiles rotate
  through an 8-buffer pool (issues self-pace via buffer-reuse waits).
- First x pair + first W arrive as interleaved k-chunks so compute can
  start as soon as ~0.8MiB lands.
- Last sample: per-m-tile copies + DMAs to shrink the tail drain.
"""

import numpy as np
import ml_dtypes

B = 64
S = 512
DIN = 1024
DH = 4096
C = 16
NCORES = 8
NSH = DH // NCORES   # 512
P = 128
KO = DIN // P        # 8
MO = S // P          # 4

LAST_RESULTS = None


def _plan(cats):
    """Largest-first category order; proc_order[i] = original sample idx."""
    first = {}
    for j, c in enumerate(cats):
        first.setdefault(c, j)
    counts = {c: cats.count(c) for c in first}
    used = sorted(first, key=lambda c: (-counts[c], first[c]))
    proc_order = [j for c in used for j in range(len(cats)) if cats[j] == c]
    group_start = {}
    for i, j in enumerate(proc_order):
        group_start.setdefault(cats[j], i)
    return used, proc_order, group_start


def _build_program(cats):
    import concourse.bacc as bacc
    import concourse.mybir as mybir
    import concourse.tile as tile

    nc = bacc.Bacc("TRN2", target_bir_lowering=False)

    assert B % 4 == 0
    x_d = nc.dram_tensor("x", (B // 4, P, 4, KO, S), mybir.dt.bfloat16,
                         kind="ExternalInput")
    w_d = nc.dram_tensor("w", (C, P, KO, NSH), mybir.dt.bfloat16,
                         kind="ExternalInput")
    out_d = nc.dram_tensor("out", (B, P, MO, NSH), mybir.dt.bfloat16,
                           kind="ExternalOutput")

    used, proc_order, group_start = _plan(cats)
    n_groups = len(used)

    # k-chunking for the first x pair and group-0 W: compute starts as
    # soon as the first ~0.4MiB lands (DMA subsystem ramps slowly).
    CHUNKS = [(0, 1), (1, 2), (2, 4), (4, 6), (6, 8)]

    with tile.TileContext(nc) as tc:
        with (
            tc.tile_pool(name="static", bufs=1) as wp,
            tc.tile_pool(name="wgroup", bufs=6) as wg,
            tc.tile_pool(name="xpool", bufs=3) as xb,
            tc.tile_pool(name="opool", bufs=3) as ob,
            tc.tile_pool(name="psum", bufs=2, space="PSUM") as ps,
        ):
            w_tiles = {}

            def emit_w(g):
                c = used[g]
                t = wg.tile([P, KO, NSH], mybir.dt.bfloat16, tag="w")
                nc.sync.dma_start(t[:], w_d[c])
                w_tiles[c] = t

            # PE warmup: dummy matmuls with no DMA deps, run during the
            # ~10us DMA-subsystem init so the clock ramps before real data.
            warm_l = wp.tile([P, P], mybir.dt.bfloat16, tag="warm_l")
            warm_r = wp.tile([P, NSH], mybir.dt.bfloat16, tag="warm_r")
            nc.any.memzero(warm_l[:])
            nc.any.memzero(warm_r[:])
            warm_p = ps.tile([P, MO, NSH], mybir.dt.float32, tag="ps")
            for _ in range(5):
                nc.tensor.matmul(warm_p[:, 0, :], warm_l[:], warm_r[:],
                                 start=True, stop=True)

            # Head DMA sequence: interleaved k-chunks of (x samples 0-1,
            # W c0), in exact need order on one queue.
            c0 = used[0]
            x0c = []
            w0c = []
            x00b = None
            for ci, (a, b) in enumerate(CHUNKS):
                if ci == 0:
                    # very first chunk: sample 0's slot only (128KB), so
                    # the first matmul starts as early as possible
                    tx = wp.tile([P, 1, b - a, S], mybir.dt.bfloat16,
                                 tag="x0c0")
                    nc.sync.dma_start(tx[:], x_d[0, :, 0:1, a:b, :])
                else:
                    tx = wp.tile([P, 2, b - a, S], mybir.dt.bfloat16,
                                 tag=f"x0c{ci}")
                    nc.sync.dma_start(tx[:], x_d[0, :, 0:2, a:b, :])
                x0c.append(tx)
                tw = wp.tile([P, b - a, NSH], mybir.dt.bfloat16,
                             tag=f"w0c{ci}")
                nc.sync.dma_start(tw[:], w_d[c0, :, a:b, :])
                w0c.append(tw)
                if ci == 0:
                    # sample 1's slot of the first k-chunk
                    x00b = wp.tile([P, 1, b - a, S], mybir.dt.bfloat16,
                                   tag="x0c0b")
                    nc.sync.dma_start(x00b[:], x_d[0, :, 1:2, a:b, :])

            def chunk_of(k):
                for ci, (a, b) in enumerate(CHUNKS):
                    if a <= k < b:
                        return ci, k - a
                raise AssertionError

            # Samples 2-3 (second half of quad 0), then whole quads.
            x0b = wp.tile([P, 2, KO, S], mybir.dt.bfloat16, tag="x0b")
            nc.sync.dma_start(x0b[:], x_d[0, :, 2:4, :, :])

            xtiles = {}

            def emit_x(q):
                t = xb.tile([P, 4, KO, S], mybir.dt.bfloat16, tag="x")
                nc.sync.dma_start(t[:], x_d[q])
                xtiles[q] = t

            emit_x(1)

            # W group issue sample: 4 samples before first use.
            w_sched = {}
            for g in range(1, n_groups):
                w_sched.setdefault(max(0, group_start[used[g]] - 4), []).append(g)

            for i in range(B):
                j = proc_order[i]
                c = cats[j]
                for g in w_sched.get(i, ()):
                    emit_w(g)
                if i % 4 == 0:
                    q = i // 4 + 2
                    if q < B // 4:
                        emit_x(q)
                pt = ps.tile([P, MO, NSH], mybir.dt.float32, tag="ps")
                ot = ob.tile([P, MO, NSH], mybir.dt.bfloat16, tag="o")

                def lhs_of(k, m):
                    if i < 2:
                        ci, kk = chunk_of(k)
                        if ci == 0:
                            t = x0c[0] if i == 0 else x00b
                            return t[:, 0, kk, m * P:(m + 1) * P]
                        return x0c[ci][:, i, kk, m * P:(m + 1) * P]
                    if i < 4:
                        return x0b[:, i - 2, k, m * P:(m + 1) * P]
                    return xtiles[i // 4][:, i % 4, k, m * P:(m + 1) * P]

                def rhs_of(k):
                    if c == c0:
                        ci, kk = chunk_of(k)
                        return w0c[ci][:, kk, :]
                    return w_tiles[c][:, k, :]

                # Samples 0-1 are chunk-arrival-gated: k-major order lets
                # each arriving k-chunk feed 4 matmuls instead of 1.
                if i < 2:
                    order = [(m, k) for k in range(KO) for m in range(MO)]
                else:
                    order = [(m, k) for m in range(MO) for k in range(KO)]
                for m, k in order:
                    nc.tensor.matmul(
                        pt[:, m, :],
                        lhs_of(k, m),
                        rhs_of(k),
                        start=(k == 0),
                        stop=(k == KO - 1),
                    )
                # Copies ride the otherwise-idle Scalar (ACT) engine.
                nc.scalar.copy(ot[:], pt[:])
                if i == B - 1:
                    # Sync's DMA queue is several times wider than GpSimd's
                    # and idle by now — shortest tail drain. (Splitting the
                    # copy across engines doesn't help: the framework
                    # serializes readers of the same PSUM tile.)
                    nc.sync.dma_start(out_d[i], ot[:])
                else:
                    nc.gpsimd.dma_start(out_d[i], ot[:])

    nc.compile()
    return nc


def kernel(x, cat_ids, W, b):
    global LAST_RESULTS
    from concourse import bass_utils

    x = np.asarray(x, dtype=np.float32)
    cat_ids_np = np.asarray(cat_ids).astype(np.int64)
    W = np.asarray(W, dtype=np.float32)
    b = np.asarray(b, dtype=np.float32)
    cats = [int(c) for c in cat_ids_np]

    used, proc_order, _ = _plan(cats)

    # x: [B,S,DIN] -> [B,P(q),KO,S] bf16, in processed order, quadded.
    xp = np.ascontiguousarray(
        x.reshape(B, S, KO, P).transpose(0, 3, 2, 1)
    ).astype(ml_dtypes.bfloat16)[proc_order]
    xp = np.ascontiguousarray(
        xp.reshape(B // 4, 4, P, KO, S).transpose(0, 2, 1, 3, 4)
    )

    in_maps = []
    for core in range(NCORES):
        Wc = W[:, :, core * NSH:(core + 1) * NSH]
        Wp = np.ascontiguousarray(
            Wc.reshape(C, KO, P, NSH).transpose(0, 2, 1, 3)
        ).astype(ml_dtypes.bfloat16)
        in_maps.append({"x": xp, "w": Wp})

    nc = _build_program(cats)
    res = bass_utils.run_bass_kernel_spmd(
        nc, in_maps, core_ids=list(range(NCORES))
    )
    LAST_RESULTS = res

    inv = np.argsort(np.asarray(proc_order))
    out = np.empty((B, S, DH), dtype=np.float32)
    for core in range(NCORES):
        oc = np.asarray(res.results[core]["out"]).astype(np.float32)
        # out[proc_order[i], m*128+p, n] = oc[i, p, m, n]
        oc = oc.reshape(B, P, MO, NSH).transpose(0, 2, 1, 3).reshape(B, S, NSH)
        out[:, :, core * NSH:(core + 1) * NSH] = oc[inv]

    if b.any():
        out += b[cats][:, None, :]
    return out
